# revision 28
# baseline (speedup 1.0000x reference)
import numpy as np

import concourse.bacc as bacc
import concourse.mybir as mybir
from concourse.bass_utils import run_bass_kernel_spmd
from concourse.tile import TileContext

N_CORES = 8
Q, UNITS, D = 2048, 512, 128
QS = Q // N_CORES
UT = UNITS // 128

DT_NAME = "fp16"
G = 32
GZ = 8
MINW = 4


def _dt():
    return mybir.dt.float16 if DT_NAME == "fp16" else mybir.dt.float32


def _np_dt():
    return np.float16 if DT_NAME == "fp16" else np.float32


def build_nc():
    dt = _dt()
    f32 = mybir.dt.float32
    nc = bacc.Bacc("TRN2", target_bir_lowering=False)
    xf = nc.dram_tensor("xf", [1, QS * D], dt, kind="ExternalInput")
    negm = nc.dram_tensor("negm", [128, UT, D], dt, kind="ExternalInput")
    rr = nc.dram_tensor("rr", [128, UT, D], dt, kind="ExternalInput")
    out = nc.dram_tensor("out", [UT, 128, QS], f32, kind="ExternalOutput")

    n_chunks = QS // G

    with TileContext(nc) as tc:
        with (
            tc.tile_pool(name="cpool", bufs=1) as cpool,
            tc.tile_pool(name="zpool", bufs=2) as zpool,
            tc.tile_pool(name="opool", bufs=1) as opool,
        ):
            xf_sb = cpool.tile([1, QS * D], dt)
            nc.sync.dma_start(xf_sb[:, :], xf[:, :])
            negm_sb = cpool.tile([128, UT, D], dt)
            nc.sync.dma_start(negm_sb[:, :, :], negm[:, :, :])
            r_sb = cpool.tile([128, UT, D], dt)
            nc.sync.dma_start(r_sb[:, :, :], rr[:, :, :])
            rc = cpool.tile([128, UT, D], dt)
            nc.vector.tensor_copy(rc[:, :, :], r_sb[:, :, :])
            nmc = cpool.tile([128, UT, D], dt)
            nc.vector.tensor_copy(nmc[:, :, :], negm_sb[:, :, :])

            osb = opool.tile([128, UT, QS], f32)

            for c in range(n_chunks):
                xb = zpool.tile([128, G * D], dt, tag="xb", name="xb")
                src = xf[0:1, c * G * D : (c + 1) * G * D].broadcast_to(
                    [128, G * D]
                )
                nc.sync.dma_start(xb[:, :], src)
                xb4 = xb.rearrange("p (g d) -> p g d", d=D).unsqueeze(1)

                zsb = zpool.tile([128, UT, G, D], dt, tag="zsb", name="zsb")
                nb = G // GZ
                for j in range(nb):
                    gs = slice(j * GZ, (j + 1) * GZ)
                    nc.vector.tensor_tensor(
                        zsb[:, :, gs, :],
                        xb4[:, :, gs, :].broadcast_to([128, UT, GZ, D]),
                        nmc.unsqueeze(2).broadcast_to([128, UT, GZ, D]),
                        mybir.AluOpType.add,
                    )
                    nc.scalar.activation(
                        out=zsb[:, :, gs, :],
                        in_=zsb[:, :, gs, :],
                        func=mybir.ActivationFunctionType.Abs,
                        scale=1.0,
                    )

                nc.vector.tensor_tensor(
                    zsb[:, :, :, :],
                    zsb[:, :, :, :],
                    rc.unsqueeze(2).broadcast_to([128, UT, G, D]),
                    mybir.AluOpType.subtract,
                )
                width = D
                while width > MINW and width % 2 == 0:
                    half = width // 2
                    nc.vector.tensor_tensor(
                        zsb[:, :, :, 0:half],
                        zsb[:, :, :, 0:half],
                        zsb[:, :, :, half:width],
                        mybir.AluOpType.max,
                    )
                    width = half
                nc.vector.tensor_reduce(
                    osb[:, :, c * G : (c + 1) * G],
                    zsb[:, :, :, 0:width],
                    axis=mybir.AxisListType.X,
                    op=mybir.AluOpType.max,
                )

            for t in range(UT):
                nc.sync.dma_start(out[t, :, :], osb[:, t, :])

    nc.compile()
    return nc


def _prep_inputs(x, Wmin, Wmax):
    ndt = _np_dt()
    m = ((Wmin + Wmax) * 0.5).astype(ndt)
    r = ((Wmax - Wmin) * 0.5).astype(ndt)
    negm_ptd = np.ascontiguousarray(
        (-m).reshape(UT, 128, D).transpose(1, 0, 2)
    )
    r_ptd = np.ascontiguousarray(r.reshape(UT, 128, D).transpose(1, 0, 2))
    xd = x.astype(ndt)
    in_maps = []
    for rnk in range(N_CORES):
        xs = np.ascontiguousarray(xd[rnk * QS : (rnk + 1) * QS].reshape(1, QS * D))
        in_maps.append({"xf": xs, "negm": negm_ptd, "rr": r_ptd})
    return in_maps


def _assemble(results):
    ys = []
    for rnk in range(N_CORES):
        o = results[rnk]["out"]
        ys.append(-o.reshape(UNITS, QS).T)
    return np.ascontiguousarray(np.concatenate(ys, axis=0).astype(np.float32))


_NC_CACHE = {}


def _get_nc():
    key = (DT_NAME, G, GZ, MINW)
    if key not in _NC_CACHE:
        _NC_CACHE[key] = build_nc()
    return _NC_CACHE[key]


def run(x, Wmin, Wmax, trace=False):
    nc = _get_nc()
    in_maps = _prep_inputs(x, Wmin, Wmax)
    res = run_bass_kernel_spmd(nc, in_maps, core_ids=list(range(N_CORES)), trace=trace)
    return _assemble(res.results), res


def kernel(x, Wmin, Wmax):
    y, _ = run(x, Wmin, Wmax, trace=False)
    return y
